# revision 1
# baseline (speedup 1.0000x reference)
"""Trainium2 Bass kernel for nn_CentersDistance (retrieval_knn).

logits[k, n] = -||centers[k] - inputs[n]||^2
             = 2*(centers @ inputs.T)[k, n] - ||centers[k]||^2 - ||inputs[n]||^2

Strategy (8 NeuronCores, data-parallel over the N=8192 inputs):
  * host: transpose both operands so the contraction dim D lands on the SBUF
    partition axis ([D, K] and [D, N/8] layouts), fold the factor 2 into the
    inputs, and precompute the norm terms exactly in float64.
  * device (per core): a 1024x1024x1024 matmul in bf16 with fp32 PSUM
    accumulation (bf16 streams 1 row/cycle on the PE vs 4 for fp32; the
    measured end-to-end error is absmax/scale 3.3e-4, resid_var 5.4e-9,
    because the exact norm terms dominate the logits).  The epilogue runs on
    the DVE: one scalar_tensor_tensor op adds -||c||^2 (per-partition scalar)
    and -||x||^2 (broadcast row read from a host-precomputed [128, N/8]
    tile), output stored fp32.
  * raw Block/semaphore implementation (not Tile): Tile's ~50 semaphores are
    not the issue (the NRT pre/postamble resets a fixed 51 per engine), but
    Tile adds its own ~6 us drain + clear-semaphores + barrier tail, and its
    scheduler cannot express the exact warmup/pacing we want.
  * the PE is kept continuously busy from ~1 us into the kernel by N_WU
    throwaway matmuls on an (uninitialized) scratch tile so the HAM clock
    gate is fully open (2.4 GHz) when the first real matmul issues; the
    warmup count is sized to bridge until the first ct/xt tile pair lands.
  * loads stream on two HW-DGE queues (Sync: xt, Scalar: ct) with one
    semaphore per d-tile pair: completions of equal-size DMAs are usually in
    issue order, but HBM contention from the other 7 cores can invert them,
    and a single shared counter would then let the PE read a tile that is
    not fully written (observed as a sporadic inf in the output).
  * pass 1 (m-tiles 0-3) runs d outermost so matmuls pace with the streaming
    loads across 8 concurrent PSUM banks; pass 2 (m-tiles 4-7) runs d
    innermost so each output group retires early and its epilogue + store
    overlap the remaining matmuls.

Measured on 8 axon-tunneled trn2 cores: ~45 us NEFF exec (NTFF), of which
~27.6 us is the bf16 PE-stream floor (128 matmuls x 512 rows @ 2.4 GHz) and
~14 us is fixed NRT preamble/postamble (sync barriers, 51-semaphore reset
chains, dma_rearm).

A float32r variant (dt=mybir.dt.float32r, np_dt=np.float32) measures
~56 us / absmax 2.0e-5 — load-bound (8.5 MB vs 4.5 MB of input) but with
near-fp32 precision; kept as a fallback should tighter accuracy ever be
needed.  An fp8e4m3 DoubleRow variant measured ~36 us / absmax 5.2e-3 —
rejected for accuracy-risk reasons.
"""

import threading
from contextlib import ExitStack

import numpy as np
import ml_dtypes

import concourse.mybir as mybir
from concourse import bacc
from concourse.bass_utils import run_bass_kernel_spmd

N_CORES = 8
N, K, D = 8192, 1024, 1024
NSH = N // N_CORES  # per-core slab of inputs
P = 128             # SBUF partitions
NF = 512            # matmul moving free dim (one fp32 PSUM bank)

D_TILES = D // P    # 8 contraction tiles
M_TILES = K // P    # 8 center tiles
H_TILES = NSH // NF # 2 moving-dim tiles

G = M_TILES * H_TILES  # 16 output groups of [128, 512]
GP1 = 8                # groups 0-7 -> pass 1 (m-tiles 0-3), banks 0-7
N_WU = 10              # PE warm-up matmuls

_DT = mybir.dt.bfloat16
_NP_DT = ml_dtypes.bfloat16

_cache = threading.local()


def _g_mh(g):
    return g // H_TILES, g % H_TILES


def _build_nc(dt=_DT):
    nc = bacc.Bacc(
        "TRN2", target_bir_lowering=False, debug=False, num_devices=N_CORES
    )
    ct = nc.dram_tensor("ct", [D, K], dt, kind="ExternalInput").ap()
    xt = nc.dram_tensor("xt", [D, NSH], dt, kind="ExternalInput").ap()
    ncsq = nc.dram_tensor(
        "ncsq", [P, M_TILES], mybir.dt.float32, kind="ExternalInput"
    ).ap()
    nxsq = nc.dram_tensor(
        "nxsq", [P, NSH], mybir.dt.float32, kind="ExternalInput"
    ).ap()
    out = nc.dram_tensor("out", [K, NSH], mybir.dt.float32, kind="ExternalOutput").ap()

    ct_r = ct.rearrange("(t p) k -> t p k", p=P)
    xt_r = xt.rearrange("(t p) n -> t p n", p=P)
    out_r = out.rearrange("(m p) n -> m p n", p=P)

    HNF = NF // 2

    with (
        nc.sbuf_tensor("wu_sb", [P, NF], dt) as wu_sb,
        nc.sbuf_tensor("ncsq_sb", [P, M_TILES], mybir.dt.float32) as ncsq_sb,
        nc.sbuf_tensor("nxsq_sb", [P, NSH], mybir.dt.float32) as nxsq_sb,
        nc.sbuf_tensor("ot_sb", [P, G * NF], mybir.dt.float32) as ot_sb,
        ExitStack() as stack,
        nc.semaphore("const_sem") as const_sem,
        nc.semaphore("mm_sem") as mm_sem,
        nc.semaphore("dve_sem") as dve_sem,
        nc.semaphore("dma_out") as dma_out,
        nc.Block() as block,
    ):
        d_sems = [
            stack.enter_context(nc.semaphore(f"d_sem{i}")) for i in range(D_TILES)
        ]
        ct_sb = [
            stack.enter_context(nc.sbuf_tensor(f"ct_sb{d}", [P, K], dt))
            for d in range(D_TILES)
        ]
        xt_sb = [
            stack.enter_context(nc.sbuf_tensor(f"xt_sb{d}", [P, NSH], dt))
            for d in range(D_TILES)
        ]
        ps = [
            stack.enter_context(nc.psum_tensor(f"ps{b}", [P, NF], mybir.dt.float32))
            for b in range(8)
        ]

        @block.sync
        def _(sync):
            # xt on the Sync HW-DGE queue; ct goes out in parallel on the
            # Scalar engine's queue (block.scalar below) — two rings halve
            # the time to the first d-tile pair and keep the d-loop ahead
            # of the PE throughout
            for d in range(D_TILES):
                sync.dma_start(xt_sb[d][:], xt_r[d]).then_inc(d_sems[d], 16)
            # consts last: only the DVE epilogue (which runs late) needs them
            sync.dma_start(ncsq_sb[:], ncsq).then_inc(const_sem, 16)
            sync.dma_start(nxsq_sb[:], nxsq).then_inc(const_sem, 16)
            for g in range(G - 1):
                m, h = _g_mh(g)
                sync.wait_ge(dve_sem, g + 1)
                sync.dma_start(
                    out_r[m][:, h * NF : (h + 1) * NF],
                    ot_sb[:, g * NF : (g + 1) * NF],
                ).then_inc(dma_out, 16)
            # last group is split in half so its store starts while the DVE
            # is still draining the second half; the second half goes out on
            # the Scalar ring (see block.scalar) so the two final stores
            # complete in parallel — both are on the kernel's critical tail
            m, h = _g_mh(G - 1)
            sync.wait_ge(dve_sem, G)
            sync.dma_start(
                out_r[m][:, h * NF : h * NF + HNF],
                ot_sb[:, (G - 1) * NF : (G - 1) * NF + HNF],
            ).then_inc(dma_out, 16)
            sync.wait_ge(dma_out, (G + 1) * 16)

        @block.scalar
        def _(scalar):
            for d in range(D_TILES):
                scalar.dma_start(ct_sb[d][:], ct_r[d]).then_inc(d_sems[d], 16)
            m, h = _g_mh(G - 1)
            scalar.wait_ge(dve_sem, G + 1)
            scalar.dma_start(
                out_r[m][:, h * NF + HNF : (h + 1) * NF],
                ot_sb[:, (G - 1) * NF + HNF : G * NF],
            ).then_inc(dma_out, 16)

        @block.tensor
        def _(tensor):
            # warm-up: open the HAM clock gate while the loads stream.
            # wu_sb is deliberately uninitialized — the products are never
            # read, only the PE-busy time matters.  Bank 7 is rewritten with
            # start=True by group 7's first matmul ~8 matmuls later, long
            # after the last warmup has drained.
            for _ in range(N_WU):
                nc.tensor.matmul(
                    ps[GP1 - 1][:], wu_sb[:, 0:P], wu_sb[:], start=True, stop=True
                )
            # pass 1: groups 0-7 accumulate in banks 0-7, d outermost so
            # matmuls pace with the streaming loads
            for d in range(D_TILES):
                tensor.wait_ge(d_sems[d], 32)
                for g in range(GP1):
                    m, h = _g_mh(g)
                    mm = nc.tensor.matmul(
                        ps[g][:],
                        ct_sb[d][:, m * P : (m + 1) * P],
                        xt_sb[d][:, h * NF : (h + 1) * NF],
                        start=(d == 0),
                        stop=(d == D_TILES - 1),
                    )
                    if d == D_TILES - 1:
                        mm.then_inc(mm_sem, 1)
            # pass 2: groups 8-15 reuse banks 0-7 once the DVE epilogue has
            # drained the pass-1 group from that bank (P10: concurrent
            # PE-write + DVE-read of one PSUM bank is fatal, so this wait is
            # load-bearing, not just WAR ordering)
            for g in range(GP1, G):
                m, h = _g_mh(g)
                if g >= 8:
                    # bank g%8 was last drained by the DVE for group g-8
                    tensor.wait_ge(dve_sem, g - 8 + 1)
                for d in range(D_TILES):
                    mm = nc.tensor.matmul(
                        ps[g % 8][:],
                        ct_sb[d][:, m * P : (m + 1) * P],
                        xt_sb[d][:, h * NF : (h + 1) * NF],
                        start=(d == 0),
                        stop=(d == D_TILES - 1),
                    )
                mm.then_inc(mm_sem, 1)

        @block.vector
        def _(vector):
            vector.wait_ge(const_sem, 32)  # ncsq + nxsq present
            for g in range(G - 1):
                m, h = _g_mh(g)
                vector.wait_ge(mm_sem, g + 1)
                nc.vector.scalar_tensor_tensor(
                    ot_sb[:, g * NF : (g + 1) * NF],
                    ps[g % 8][:],
                    ncsq_sb[:, m : m + 1],
                    nxsq_sb[:, h * NF : (h + 1) * NF],
                    op0=mybir.AluOpType.add,
                    op1=mybir.AluOpType.add,
                ).then_inc(dve_sem, 1)
            m, h = _g_mh(G - 1)
            vector.wait_ge(mm_sem, G)
            for half in range(2):
                nc.vector.scalar_tensor_tensor(
                    ot_sb[
                        :,
                        (G - 1) * NF + half * HNF : (G - 1) * NF + (half + 1) * HNF,
                    ],
                    ps[(G - 1) % 8][:, half * HNF : (half + 1) * HNF],
                    ncsq_sb[:, m : m + 1],
                    nxsq_sb[:, h * NF + half * HNF : h * NF + (half + 1) * HNF],
                    op0=mybir.AluOpType.add,
                    op1=mybir.AluOpType.add,
                ).then_inc(dve_sem, 1)

    nc.compile()
    return nc


def _get_nc():
    if not hasattr(_cache, "nc"):
        _cache.nc = _build_nc()
    return _cache.nc


def kernel(inputs, centers, _trace=False, _np_dt=None):
    np_dt = _np_dt if _np_dt is not None else _NP_DT
    inputs = np.asarray(inputs, dtype=np.float32)
    centers = np.asarray(centers, dtype=np.float32)

    csq = np.sum(centers.astype(np.float64) ** 2, axis=1)
    xsq = np.sum(inputs.astype(np.float64) ** 2, axis=1)

    ct = np.ascontiguousarray(centers.T).astype(np_dt)
    xt2 = np.ascontiguousarray((2.0 * inputs).T.astype(np_dt))
    ncsq = np.ascontiguousarray((-csq).reshape(M_TILES, P).T.astype(np.float32))

    in_maps = []
    for i in range(N_CORES):
        sl = slice(i * NSH, (i + 1) * NSH)
        in_maps.append(
            {
                "ct": ct,
                "xt": np.ascontiguousarray(xt2[:, sl]),
                "ncsq": ncsq,
                "nxsq": np.ascontiguousarray(
                    np.broadcast_to(-xsq[sl], (P, NSH))
                ).astype(np.float32),
            }
        )

    nc = _get_nc()
    try:
        res = run_bass_kernel_spmd(
            nc, in_maps, core_ids=list(range(N_CORES)), trace=_trace
        )
    except ModuleNotFoundError:
        # NTFF trace glue is absent in some images; rerun without tracing
        res = run_bass_kernel_spmd(
            nc, in_maps, core_ids=list(range(N_CORES)), trace=False
        )
    if _trace:
        kernel.last_results = res
    return np.concatenate([r["out"] for r in res.results], axis=1)



# revision 2
# speedup vs baseline: 1.2535x; 1.2535x over previous
"""Trainium2 Bass kernel for nn_CentersDistance (retrieval_knn).

logits[k, n] = -||centers[k] - inputs[n]||^2
             = 2*(centers @ inputs.T)[k, n] - ||centers[k]||^2 - ||inputs[n]||^2

Strategy (8 NeuronCores, data-parallel over the N=8192 inputs):
  * host: transpose both operands so the contraction dim D lands on the SBUF
    partition axis, fold the factor 2 into the inputs, quantize both to
    fp8e4m3 (TRN float8e4), and precompute the norm terms exactly in float64.
  * device (per core): a 1024x1024x1024 matmul in fp8 with DoubleRow perf
    mode: each InstMatmult consumes TWO 128-deep contraction tiles laid out
    as [128, 2, free] (2 rows/cycle on the PE = 157 TF/s, 2x the bf16 rate),
    so the whole GEMM is 64 matmul instructions instead of 128.  PSUM
    accumulation stays fp32.  The epilogue runs on the DVE: one
    scalar_tensor_tensor op adds -||c||^2 (per-partition scalar) and
    -||x||^2 (broadcast row), output stored bf16 (halves the store traffic;
    host converts back to fp32).
  * -||x||^2 is shipped as a single [1, 1024] fp32 row (4 KB) and broadcast
    to all 128 partitions on-chip by the otherwise-idle GpSimd engine
    (partition_broadcast), replacing the baseline's 512 KB host-broadcast
    load.
  * raw Block/semaphore implementation (not Tile), same skeleton as the
    bf16 baseline: two HW-DGE queues (Sync: xt, Scalar: ct) with one
    semaphore per d-pair; PE warmup matmuls bridge the NRT preamble until
    the first tile pair lands and open the HAM clock gate; pass 1 (m-tiles
    0-3) runs d outermost to pace with the streaming loads across 8 PSUM
    banks; pass 2 (m-tiles 4-7) runs d innermost so each output group
    retires early and its epilogue + store overlap the remaining matmuls.
  * stores pair adjacent groups (same m-tile) into single [128, 1024] bf16
    DMAs (2 KB/partition lines) alternating between the two queues; the
    last group is split in half across both queues to shorten the tail.

Accuracy: the exact f64 norm terms dominate the logits, so the fp8 cross
term + bf16 store land at absmax/scale ~6e-3 vs the 2e-2 gate (the bf16
baseline measured 3.3e-4; kept in kernel_bf16_baseline.py as fallback).
"""

import threading
from contextlib import ExitStack

import numpy as np
import ml_dtypes

import concourse.mybir as mybir
from concourse import bacc
from concourse.bass_utils import run_bass_kernel_spmd

N_CORES = 8
N, K, D = 8192, 1024, 1024
NSH = N // N_CORES  # per-core slab of inputs
P = 128             # SBUF partitions
NF = 512            # matmul moving free dim (one fp32 PSUM bank)

DP_TILES = D // (2 * P)  # 4 double-row contraction tiles (256 deep each)
M_TILES = K // P         # 8 center tiles
H_TILES = NSH // NF      # 2 moving-dim tiles

G = M_TILES * H_TILES  # 16 output groups of [128, 512]
GP1 = 8                # groups 0-7 -> pass 1 (m-tiles 0-3), banks 0-7
N_WU = 4               # PE warm-up matmuls

_DT = mybir.dt.float8e4
_NP_DT = ml_dtypes.float8_e4m3
_OUT_DT = mybir.dt.bfloat16
_DR = mybir.MatmulPerfMode.DoubleRow

_cache = threading.local()


def _g_mh(g):
    return g // H_TILES, g % H_TILES


def _build_nc():
    nc = bacc.Bacc(
        "TRN2", target_bir_lowering=False, debug=False, num_devices=N_CORES
    )
    ct = nc.dram_tensor("ct", [DP_TILES, P, 2, K], _DT, kind="ExternalInput").ap()
    xt = nc.dram_tensor("xt", [DP_TILES, P, 2, NSH], _DT, kind="ExternalInput").ap()
    ncsq = nc.dram_tensor(
        "ncsq", [P, M_TILES], mybir.dt.float32, kind="ExternalInput"
    ).ap()
    nxrow = nc.dram_tensor(
        "nxrow", [1, NSH], mybir.dt.float32, kind="ExternalInput"
    ).ap()
    out = nc.dram_tensor("out", [K, NSH], _OUT_DT, kind="ExternalOutput").ap()

    out_r = out.rearrange("(m p) n -> m p n", p=P)

    HNF = NF // 2

    with (
        nc.sbuf_tensor("wu_sb", [P, 2, NF], _DT) as wu_sb,
        nc.sbuf_tensor("ncsq_sb", [P, M_TILES], mybir.dt.float32) as ncsq_sb,
        nc.sbuf_tensor("nxrow_sb", [1, NSH], mybir.dt.float32) as nxrow_sb,
        nc.sbuf_tensor("nxsq_sb", [P, NSH], mybir.dt.float32) as nxsq_sb,
        nc.sbuf_tensor("ot_sb", [P, G * NF], _OUT_DT) as ot_sb,
        ExitStack() as stack,
        nc.semaphore("row_sem") as row_sem,
        nc.semaphore("const_sem") as const_sem,
        nc.semaphore("bc_sem") as bc_sem,
        nc.semaphore("mm_sem") as mm_sem,
        nc.semaphore("dve_sem") as dve_sem,
        nc.semaphore("dma_out") as dma_out,
        nc.Block() as block,
    ):
        d_sems = [
            stack.enter_context(nc.semaphore(f"d_sem{i}")) for i in range(DP_TILES)
        ]
        ct_sb = [
            stack.enter_context(nc.sbuf_tensor(f"ct_sb{d}", [P, 2, K], _DT))
            for d in range(DP_TILES)
        ]
        xt_sb = [
            stack.enter_context(nc.sbuf_tensor(f"xt_sb{d}", [P, 2, NSH], _DT))
            for d in range(DP_TILES)
        ]
        ps = [
            stack.enter_context(nc.psum_tensor(f"ps{b}", [P, NF], mybir.dt.float32))
            for b in range(8)
        ]

        # store schedule: pair adjacent groups (same m-tile -> contiguous in
        # out) into one [128, 1024] bf16 DMA with 2KB/partition lines.
        # Pairs alternate between the Sync and Scalar queues.  The final two
        # groups stay unpaired: group 14 goes out whole, group 15 is split
        # in half across both queues so the two final stores complete in
        # parallel on the kernel's critical tail.
        def paired_stores(eng, parity):
            for gp in range(7):  # pairs (0,1) (2,3) ... (12,13)
                if gp % 2 != parity:
                    continue
                g0 = 2 * gp
                m, _ = _g_mh(g0)
                eng.wait_ge(dve_sem, g0 + 2)
                eng.dma_start(
                    out_r[m][:],
                    ot_sb[:, g0 * NF : (g0 + 2) * NF],
                ).then_inc(dma_out, 16)

        N_STORES = 7 + 1 + 2  # 7 pairs + group 14 + two halves of group 15

        @block.sync
        def _(sync):
            # xt on the Sync HW-DGE queue; ct goes out in parallel on the
            # Scalar engine's queue (block.scalar below)
            for d in range(DP_TILES):
                sync.dma_start(xt_sb[d][:], xt[d]).then_inc(d_sems[d], 16)
            # consts after the matmul-critical tiles: the GpSimd broadcast
            # and DVE epilogue need them only ~8us later
            sync.dma_start(nxrow_sb[:], nxrow).then_inc(row_sem, 16)
            sync.dma_start(ncsq_sb[:], ncsq).then_inc(const_sem, 16)
            paired_stores(sync, 0)
            # group 14 whole
            m, h = _g_mh(G - 2)
            sync.wait_ge(dve_sem, G - 1)
            sync.dma_start(
                out_r[m][:, h * NF : (h + 1) * NF],
                ot_sb[:, (G - 2) * NF : (G - 1) * NF],
            ).then_inc(dma_out, 16)
            # first half of group 15
            m, h = _g_mh(G - 1)
            sync.wait_ge(dve_sem, G)
            sync.dma_start(
                out_r[m][:, h * NF : h * NF + HNF],
                ot_sb[:, (G - 1) * NF : (G - 1) * NF + HNF],
            ).then_inc(dma_out, 16)
            sync.wait_ge(dma_out, N_STORES * 16)

        @block.scalar
        def _(scalar):
            for d in range(DP_TILES):
                scalar.dma_start(ct_sb[d][:], ct[d]).then_inc(d_sems[d], 16)
            paired_stores(scalar, 1)
            # second half of group 15
            m, h = _g_mh(G - 1)
            scalar.wait_ge(dve_sem, G + 1)
            scalar.dma_start(
                out_r[m][:, h * NF + HNF : (h + 1) * NF],
                ot_sb[:, (G - 1) * NF + HNF : G * NF],
            ).then_inc(dma_out, 16)

        @block.gpsimd
        def _(gpsimd):
            gpsimd.wait_ge(row_sem, 16)
            nc.gpsimd.partition_broadcast(nxsq_sb[:], nxrow_sb[:]).then_inc(
                bc_sem, 1
            )

        @block.tensor
        def _(tensor):
            # warm-up: open the HAM clock gate while the loads stream.
            # wu_sb is deliberately uninitialized - the products are never
            # read, only the PE-busy time matters.  Bank 7 is rewritten with
            # start=True by group 7's first matmul ~8 matmuls later.
            for _ in range(N_WU):
                nc.tensor.matmul(
                    ps[GP1 - 1][:],
                    wu_sb[:, :, 0:P],
                    wu_sb[:, :, :],
                    start=True,
                    stop=True,
                    perf_mode=_DR,
                )
            # pass 1: groups 0-7 accumulate in banks 0-7, d outermost so
            # matmuls pace with the streaming loads
            for d in range(DP_TILES):
                tensor.wait_ge(d_sems[d], 32)
                for g in range(GP1):
                    m, h = _g_mh(g)
                    mm = nc.tensor.matmul(
                        ps[g][:],
                        ct_sb[d][:, :, m * P : (m + 1) * P],
                        xt_sb[d][:, :, h * NF : (h + 1) * NF],
                        start=(d == 0),
                        stop=(d == DP_TILES - 1),
                        perf_mode=_DR,
                    )
                    if d == DP_TILES - 1:
                        mm.then_inc(mm_sem, 1)
            # pass 2: groups 8-15 reuse banks 0-7 once the DVE epilogue has
            # drained the pass-1 group from that bank (P10: concurrent
            # PE-write + DVE-read of one PSUM bank is fatal, so this wait is
            # load-bearing, not just WAR ordering)
            for g in range(GP1, G):
                m, h = _g_mh(g)
                tensor.wait_ge(dve_sem, g - 8 + 1)
                for d in range(DP_TILES):
                    mm = nc.tensor.matmul(
                        ps[g % 8][:],
                        ct_sb[d][:, :, m * P : (m + 1) * P],
                        xt_sb[d][:, :, h * NF : (h + 1) * NF],
                        start=(d == 0),
                        stop=(d == DP_TILES - 1),
                        perf_mode=_DR,
                    )
                mm.then_inc(mm_sem, 1)

        @block.vector
        def _(vector):
            vector.wait_ge(const_sem, 16)  # ncsq present
            vector.wait_ge(bc_sem, 1)      # nxsq broadcast done
            for g in range(G - 1):
                m, h = _g_mh(g)
                vector.wait_ge(mm_sem, g + 1)
                nc.vector.scalar_tensor_tensor(
                    ot_sb[:, g * NF : (g + 1) * NF],
                    ps[g % 8][:],
                    ncsq_sb[:, m : m + 1],
                    nxsq_sb[:, h * NF : (h + 1) * NF],
                    op0=mybir.AluOpType.add,
                    op1=mybir.AluOpType.add,
                ).then_inc(dve_sem, 1)
            # last group in two halves so its store starts while the DVE is
            # still draining the second half
            m, h = _g_mh(G - 1)
            vector.wait_ge(mm_sem, G)
            for half in range(2):
                nc.vector.scalar_tensor_tensor(
                    ot_sb[
                        :,
                        (G - 1) * NF + half * HNF : (G - 1) * NF + (half + 1) * HNF,
                    ],
                    ps[(G - 1) % 8][:, half * HNF : (half + 1) * HNF],
                    ncsq_sb[:, m : m + 1],
                    nxsq_sb[:, h * NF + half * HNF : h * NF + (half + 1) * HNF],
                    op0=mybir.AluOpType.add,
                    op1=mybir.AluOpType.add,
                ).then_inc(dve_sem, 1)

    nc.compile()
    return nc


def _get_nc():
    if not hasattr(_cache, "nc"):
        _cache.nc = _build_nc()
    return _cache.nc


def _to_dr_layout(a_t):
    """[D, F] -> [DP_TILES, P, 2, F]: d = dp*256 + i*128 + p."""
    F = a_t.shape[1]
    return np.ascontiguousarray(
        a_t.reshape(DP_TILES, 2, P, F).transpose(0, 2, 1, 3)
    )


def kernel(inputs, centers, _trace=False):
    inputs = np.asarray(inputs, dtype=np.float32)
    centers = np.asarray(centers, dtype=np.float32)

    csq = np.sum(centers.astype(np.float64) ** 2, axis=1)
    xsq = np.sum(inputs.astype(np.float64) ** 2, axis=1)

    ct8 = _to_dr_layout(centers.T.astype(_NP_DT))
    xt8_full = (2.0 * inputs).T.astype(_NP_DT)  # [D, N]
    ncsq = np.ascontiguousarray((-csq).reshape(M_TILES, P).T.astype(np.float32))

    in_maps = []
    for i in range(N_CORES):
        sl = slice(i * NSH, (i + 1) * NSH)
        in_maps.append(
            {
                "ct": ct8,
                "xt": _to_dr_layout(xt8_full[:, sl]),
                "ncsq": ncsq,
                "nxrow": np.ascontiguousarray(
                    (-xsq[sl]).reshape(1, NSH).astype(np.float32)
                ),
            }
        )

    nc = _get_nc()
    try:
        res = run_bass_kernel_spmd(
            nc, in_maps, core_ids=list(range(N_CORES)), trace=_trace
        )
    except ModuleNotFoundError:
        # NTFF trace glue is absent in some images; rerun without tracing
        res = run_bass_kernel_spmd(
            nc, in_maps, core_ids=list(range(N_CORES)), trace=False
        )
    if _trace:
        kernel.last_results = res
    return np.concatenate(
        [np.asarray(r["out"]).astype(np.float32) for r in res.results], axis=1
    )
